# revision 54
# baseline (speedup 1.0000x reference)
"""LocallyConnected2d (3x3, stride 1, pad 1) Trainium2 kernel, 8-way spatial-parallel.

out[n,o,h,w] = sum_{c,i,k} weight[o,h,w,c,i,k] * xpad[n,c,h+i,w+k] + bias[o,h,w]

Sharding: output rows h are split 7-per-core across 8 NeuronCores. Each core
streams its private 1/8 weight slice exactly once, in bf16 (~7.2MB, the
dominant traffic; the all-zero padded border columns are not shipped).

x rows are loaded once (1.07MB) as three 3-row tiles; the 3-row halo tiles for
rows not aligned to a tile boundary (h = 1, 2, 4, 5) are assembled on-chip by
32-partition-offset DVE copies that overlap earlier rows' matmuls.

Per output row h and padded input column j (1..56), the contraction over
(i, c) = 96 terms is one bf16 matmul: lhsT = x column block [96, n=32]
(stationary), rhs = per-pixel weights [96, (pixel, o) <= 96] (moving),
accumulated in fp32 PSUM over the 3 columns j = w..w+2 that feed each output
pixel w. PSUM groups are zero-initialized by DVE memsets so the tensor engine
runs only real contraction work; bias (zeros in this problem) is added on
host. All input loads ride the sync HWDGE ring, issued up front wait-free, in
half-row weight chunks ordered so row 0 can start immediately; outputs are
evicted by the scalar engine as bf16 into a [128, 448] row tile (partition =
(group, n)) and leave in one 128-partition DMA per row on the scalar ring.
The output is transposed to NCHW on host, where the bias is added.
"""

import numpy as np
from ml_dtypes import bfloat16

import concourse.bass as bass
import concourse.mybir as mybir
import concourse.tile as tile
from concourse.vector_clock import ScopedClock, VectorClock
from concourse.bass_utils import run_bass_kernel_spmd

N, C, H, W = 32, 32, 56, 56
O = 32
NCORES = 8
R = H // NCORES          # output rows per core
JW = W + 2               # padded input columns
NJ = W                   # shipped weight columns (j = 1..56; 0 and 57 are dead)
JSPLIT = 30              # weight chunk A covers j=1..29, chunk B j=30..56
GP = 14                  # pixels per PSUM group (14*32 = 448 <= 512 fp32/bank)
NG = W // GP
KP = 3 * C               # contraction partitions: (i, c)

_patched = False


def _patch_tile_drain():
    """The walrus build in this container rejects >1 sem wait on an InstDrain.
    Move the Tile tail-drain's waits onto one sync-engine nop per processor
    (same-engine in-order issue makes this equivalent), leaving the drain bare.
    """
    global _patched
    if _patched:
        return

    def _drain_and_barrier(self, tick_clock, wait_clock):
        # The stock tail is two all-engine EVSEM butterflies (~27 serial
        # event-semaphore waits per engine each, ~10us of pure drain) around
        # the semaphore cleanup. The barriers only exist to order the
        # gpsimd-issued cleanup after all work, so instead: wait for every
        # logical processor's final vector-clock tick directly on gpsimd
        # nops, then clean up. Every other engine just drains and halts; the
        # NEFF ends when gpsimd finishes the cleanup.
        gc = tick_clock.global_clock
        n = len(gc)
        for proc in range(n):
            t = gc[proc]
            if t <= 0:
                continue
            vec = [0] * n
            vec[proc] = t
            nop = self.nc.gpsimd.nop(nofuse=True)
            wait_clock.add_sem_waits(nop.ins, ScopedClock({None: VectorClock(vec)}))
        for eng in self.nc.engines.values():
            eng.drain()
        assert self.sems is not None
        popped = self.nc._tile_sem_poison_stack.pop()
        assert popped is self._sem_poison
        self.nc.clear_and_free_semaphores(list(self.sems.allocated().values()))

    tile.TileContext._drain_and_barrier = _drain_and_barrier
    _patched = True


def _split_multi_waits(nc):
    """This container's walrus accepts at most one semaphore wait per lowered
    instruction (matmul waits land on its single-slot LDWEIGHTS). Hoist all
    but the last wait of every instruction onto same-engine NoOps just before
    it; same-engine in-order issue preserves the wait semantics."""
    ctr = 0
    for fn in nc.m.functions:
        for bb in fn.blocks:
            out = []
            for inst in bb.instructions:
                si = inst.sync_info
                if si is not None and len(si.on_wait) > 1:
                    waits = list(si.on_wait)
                    for w in waits[:-1]:
                        ctr += 1
                        nop = mybir.InstNoOp(
                            name=f"{inst.name}-wsplit-{ctr}",
                            sync_info=mybir.SyncInfo(on_wait=[w], on_update=[]),
                            bass_nofuse=True,
                            engine=inst.engine,
                        )
                        out.append(nop)
                    si.on_wait = [waits[-1]]
                out.append(inst)
            bb.instructions = out
    return ctr


_nc_cache = None


def _build_nc():
    global _nc_cache
    if _nc_cache is not None:
        return _nc_cache
    _patch_tile_drain()
    nc = bass.Bass()
    f32 = mybir.dt.float32
    bf16 = mybir.dt.bfloat16
    NA = JSPLIT - 1            # chunk A columns (j=1..29)
    NB = NJ - NA               # chunk B columns (j=30..56)
    wta = nc.dram_tensor("wta", [R, KP, NA * 3 * O], bf16, kind="ExternalInput")
    wtb = nc.dram_tensor("wtb", [R, KP, NB * 3 * O], bf16, kind="ExternalInput")
    xh = nc.dram_tensor("xh", [3, KP, JW * N], bf16, kind="ExternalInput")
    out = nc.dram_tensor("out", [R, NG * N, GP * O], bf16, kind="ExternalOutput")

    with tile.TileContext(nc) as tc:
        with (
            tc.tile_pool(name="xb", bufs=3) as xbase,
            tc.tile_pool(name="xa", bufs=4) as xasm,
            tc.tile_pool(name="wa", bufs=R) as wpa,
            tc.tile_pool(name="wb", bufs=R) as wpb,
            tc.tile_pool(name="op", bufs=3) as opool,
            tc.tile_pool(name="ps", bufs=8, space="PSUM") as pspool,
        ):
            # All loads ride the sync ring, issued up front with no waits so
            # the sequencer never stalls and the ring stays dense. Ring order
            # is chosen so the PE can start as early as possible: row 0's x
            # halo and weight chunks first, then the remaining x tiles, then
            # the rest of the weight stream.
            xb = [xbase.tile([KP, JW * N], bf16, name="xb", tag="xb") for _ in range(3)]
            was, wbs = [], []
            for h in range(R):
                was.append(wpa.tile([KP, NA * 3 * O], bf16, name="wta_t", tag="wa"))
                wbs.append(wpb.tile([KP, NB * 3 * O], bf16, name="wtb_t", tag="wb"))
            nc.sync.dma_start(out=xb[0], in_=xh[0])
            nc.sync.dma_start(out=was[0], in_=wta[0])
            nc.sync.dma_start(out=wbs[0], in_=wtb[0])
            nc.sync.dma_start(out=xb[1], in_=xh[1])
            nc.sync.dma_start(out=xb[2], in_=xh[2])
            for h in range(1, R):
                nc.sync.dma_start(out=was[h], in_=wta[h])
                nc.sync.dma_start(out=wbs[h], in_=wtb[h])

            # Assemble halo tiles for h = 1, 2, 4, 5 from the base tiles via
            # partition-offset copies on gpsimd (idle otherwise). h=3p+r
            # needs base[p][32r:96] in partitions [0:96-32r] and
            # base[p+1][0:32r] in partitions [96-32r:96].
            xt_of = {0: xb[0], 3: xb[1], 6: xb[2]}
            def _assemble(h):
                p, rr = divmod(h, 3)
                x_t = xasm.tile([KP, JW * N], bf16)
                # one 32-partition copy per halo row (BIR: offset partition
                # windows may span at most 32 partitions); DVE bf16 copies
                # (Act ACTIVATE copies here overload the scalar engine)
                for i in range(3):
                    sp, sr = divmod(rr + i, 3)
                    nc.vector.tensor_copy(
                        out=x_t[32 * i : 32 * (i + 1), :],
                        in_=xb[p + sp][32 * sr : 32 * (sr + 1), :],
                    )
                xt_of[h] = x_t

            for h in range(R):
                x_t = xt_of[h]
                w_a = was[h]
                w_b = wbs[h]
                orow = opool.tile([NG * N, GP * O], bf16)
                for g in range(NG):
                    wa = g * GP
                    ps = pspool.tile([N, GP * O], f32)
                    nc.vector.memset(ps, 0.0)
                    # padded x columns 0 and 57 are all-zero: skipped
                    jlist = [j for j in range(wa, wa + GP + 2) if 0 < j < JW - 1]
                    for j in jlist:
                        lo = max(j - 2, wa)
                        hi = min(j, wa + GP - 1)
                        wlo = lo - (j - 2)
                        nwin = hi - lo + 1
                        if j < JSPLIT:
                            rhs = w_a[:, (j - 1) * 96 + wlo * O :]
                        else:
                            rhs = w_b[:, (j - JSPLIT) * 96 + wlo * O :]
                        nc.tensor.matmul(
                            ps[:, (lo - wa) * O : (lo - wa + nwin) * O],
                            lhsT=x_t[:, j * N : (j + 1) * N],
                            rhs=rhs[:, : nwin * O],
                            start=False,
                            stop=(j == jlist[-1]),
                            skip_group_check=True,
                        )
                    # evict bank g (fp32) as bf16 into the 128-partition row
                    # tile at partition offset 32*g (straight copy, no reorder)
                    nc.scalar.copy(out=orow[g * N : (g + 1) * N, :], in_=ps)
                nc.scalar.dma_start(out=out[h], in_=orow)
                # emit halo assembly after a row's work so the copies overlap
                # that row's matmuls instead of blocking its PSUM inits
                if h == 0:
                    _assemble(1)
                    _assemble(2)
                elif h == 2:
                    _assemble(4)
                    _assemble(5)

    _split_multi_waits(nc)
    _nc_cache = nc
    return nc


def _pack_core(weight, xp, core):
    h0 = core * R
    Wc = weight[:, h0 : h0 + R]  # [O, R, W, C, 3, 3]
    wtc = np.zeros((3, C, R, JW, 3, O), np.float32)
    for wp in range(3):
        k = 2 - wp
        src = Wc[:, :, :, :, :, k]  # [O, R, W, C, I]
        wtc[:, :, :, 2 - wp : 2 - wp + W, wp, :] = src.transpose(4, 3, 1, 2, 0)
    # [R, (i,c), (j, s, o)], border columns j=0 and j=57 dropped; split into
    # two contiguous per-h chunks at j=JSPLIT for fine-grained streaming
    wtc = wtc.transpose(2, 0, 1, 3, 4, 5).reshape(R, KP, JW, 3 * O)
    wta = np.ascontiguousarray(
        wtc[:, :, 1:JSPLIT].reshape(R, KP, -1)
    ).astype(bfloat16)
    wtb = np.ascontiguousarray(
        wtc[:, :, JSPLIT : JW - 1].reshape(R, KP, -1)
    ).astype(bfloat16)
    # x: padded rows h0..h0+8 as three 3-row tiles [(r, c), (j, n)]
    xhc = (
        xp[:, :, h0 : h0 + R + 2, :]
        .transpose(2, 1, 3, 0)
        .reshape(3, KP, JW * N)
    )
    return {
        "wta": wta,
        "wtb": wtb,
        "xh": np.ascontiguousarray(xhc).astype(bfloat16),
    }


def kernel(x, weight, bias, _want_trace=False):
    x = np.asarray(x, dtype=np.float32)
    weight = np.asarray(weight, dtype=np.float32)
    bias = np.asarray(bias, dtype=np.float32)
    nc = _build_nc()
    xp = np.pad(x, ((0, 0), (0, 0), (1, 1), (1, 1)))
    in_maps = [_pack_core(weight, xp, c) for c in range(NCORES)]
    res = run_bass_kernel_spmd(
        nc, in_maps, core_ids=list(range(NCORES)), trace=_want_trace
    )
    outs = []
    for i in range(NCORES):
        o = res.results[i]["out"].astype(np.float32)  # [R, (g, n), (w', o)]
        o = (
            o.reshape(R, NG, N, GP, O)
            .transpose(2, 4, 0, 1, 3)
            .reshape(N, O, R, W)
        )
        outs.append(o)
    full = np.concatenate(outs, axis=2) + bias
    if _want_trace:
        return full, res
    return full


# revision 55
# speedup vs baseline: 1.0885x; 1.0885x over previous
"""LocallyConnected2d (3x3, stride 1, pad 1) Trainium2 kernel, 8-way spatial-parallel.

out[n,o,h,w] = sum_{c,i,k} weight[o,h,w,c,i,k] * xpad[n,c,h+i,w+k] + bias[o,h,w]

Sharding: output rows h are split 7-per-core across 8 NeuronCores. Each core
streams its private 1/8 weight slice exactly once, in bf16 (~7.2MB, the
dominant traffic; the all-zero padded border columns are not shipped).

x rows are loaded once (1.07MB) as three 3-row tiles; the 3-row halo tiles for
rows not aligned to a tile boundary (h = 1, 2, 4, 5) are assembled on-chip by
32-partition-offset DVE copies that overlap earlier rows' matmuls.

Per output row h and padded input column j (1..56), the contraction over
(i, c) = 96 terms is one bf16 matmul: lhsT = x column block [96, n=32]
(stationary), rhs = per-pixel weights [96, (pixel, o) <= 96] (moving),
accumulated in fp32 PSUM over the 3 columns j = w..w+2 that feed each output
pixel w. PSUM groups are zero-initialized by DVE memsets so the tensor engine
runs only real contraction work; bias (zeros in this problem) is added on
host. All input loads ride the sync HWDGE ring, issued up front wait-free, in
half-row weight chunks ordered so row 0 can start immediately; outputs are
evicted by the scalar engine as bf16 into a [128, 448] row tile (partition =
(group, n)) and leave in one 128-partition DMA per row on the scalar ring.
The output is transposed to NCHW on host, where the bias is added.
"""

import numpy as np
from ml_dtypes import bfloat16

import concourse.bass as bass
import concourse.mybir as mybir
import concourse.tile as tile
from concourse.vector_clock import ScopedClock, VectorClock
from concourse.bass_utils import run_bass_kernel_spmd

N, C, H, W = 32, 32, 56, 56
O = 32
NCORES = 8
R = H // NCORES          # output rows per core
JW = W + 2               # padded input columns
NJ = W                   # shipped weight columns (j = 1..56; 0 and 57 are dead)
JSPLIT = 30              # weight chunk A covers j=1..29, chunk B j=30..56
GP = 14                  # pixels per PSUM group (14*32 = 448 <= 512 fp32/bank)
NG = W // GP
KP = 3 * C               # contraction partitions: (i, c)

_patched = False


def _patch_tile_drain():
    """The walrus build in this container rejects >1 sem wait on an InstDrain.
    Move the Tile tail-drain's waits onto one sync-engine nop per processor
    (same-engine in-order issue makes this equivalent), leaving the drain bare.
    """
    global _patched
    if _patched:
        return

    def _drain_and_barrier(self, tick_clock, wait_clock):
        # The stock tail is two all-engine EVSEM butterflies (~27 serial
        # event-semaphore waits per engine each, ~10us of pure drain) around
        # the semaphore cleanup. The barriers only exist to order the
        # gpsimd-issued cleanup after all work, so instead: wait for every
        # logical processor's final vector-clock tick directly on gpsimd
        # nops, then clean up. Every other engine just drains and halts; the
        # NEFF ends when gpsimd finishes the cleanup.
        gc = tick_clock.global_clock
        n = len(gc)
        for proc in range(n):
            t = gc[proc]
            if t <= 0:
                continue
            vec = [0] * n
            vec[proc] = t
            nop = self.nc.gpsimd.nop(nofuse=True)
            wait_clock.add_sem_waits(nop.ins, ScopedClock({None: VectorClock(vec)}))
        for eng in self.nc.engines.values():
            eng.drain()
        assert self.sems is not None
        popped = self.nc._tile_sem_poison_stack.pop()
        assert popped is self._sem_poison
        self.nc.clear_and_free_semaphores(list(self.sems.allocated().values()))

    tile.TileContext._drain_and_barrier = _drain_and_barrier
    _patched = True


def _split_multi_waits(nc):
    """This container's walrus accepts at most one semaphore wait per lowered
    instruction (matmul waits land on its single-slot LDWEIGHTS). Hoist all
    but the last wait of every instruction onto same-engine NoOps just before
    it; same-engine in-order issue preserves the wait semantics."""
    ctr = 0
    for fn in nc.m.functions:
        for bb in fn.blocks:
            out = []
            for inst in bb.instructions:
                si = inst.sync_info
                if si is not None and len(si.on_wait) > 1:
                    waits = list(si.on_wait)
                    for w in waits[:-1]:
                        ctr += 1
                        nop = mybir.InstNoOp(
                            name=f"{inst.name}-wsplit-{ctr}",
                            sync_info=mybir.SyncInfo(on_wait=[w], on_update=[]),
                            bass_nofuse=True,
                            engine=inst.engine,
                        )
                        out.append(nop)
                    si.on_wait = [waits[-1]]
                out.append(inst)
            bb.instructions = out
    return ctr


_nc_cache = None


def _build_nc():
    global _nc_cache
    if _nc_cache is not None:
        return _nc_cache
    _patch_tile_drain()
    nc = bass.Bass()
    f32 = mybir.dt.float32
    bf16 = mybir.dt.bfloat16
    NA = JSPLIT - 1            # chunk A columns (j=1..29)
    NB = NJ - NA               # chunk B columns (j=30..56)
    wta = nc.dram_tensor("wta", [R, KP, NA * 3 * O], bf16, kind="ExternalInput")
    wtb = nc.dram_tensor("wtb", [R, KP, NB * 3 * O], bf16, kind="ExternalInput")
    xh = nc.dram_tensor("xh", [3, KP, JW * N], bf16, kind="ExternalInput")
    out = nc.dram_tensor("out", [R, NG * N, GP * O], bf16, kind="ExternalOutput")

    with tile.TileContext(nc) as tc:
        with (
            tc.tile_pool(name="xb", bufs=3) as xbase,
            tc.tile_pool(name="xa", bufs=4) as xasm,
            tc.tile_pool(name="wa", bufs=R) as wpa,
            tc.tile_pool(name="wb", bufs=R) as wpb,
            tc.tile_pool(name="op", bufs=3) as opool,
            tc.tile_pool(name="ps", bufs=8, space="PSUM") as pspool,
        ):
            # All loads ride the sync ring, issued up front with no waits so
            # the sequencer never stalls and the ring stays dense. Ring order
            # is chosen so the PE can start as early as possible: row 0's x
            # halo and weight chunks first, then the remaining x tiles, then
            # the rest of the weight stream.
            xb = [xbase.tile([KP, JW * N], bf16, name="xb", tag="xb") for _ in range(3)]
            was, wbs = [], []
            for h in range(R):
                was.append(wpa.tile([KP, NA * 3 * O], bf16, name="wta_t", tag="wa"))
                wbs.append(wpb.tile([KP, NB * 3 * O], bf16, name="wtb_t", tag="wb"))
            nc.sync.dma_start(out=xb[0], in_=xh[0])
            nc.sync.dma_start(out=was[0], in_=wta[0])
            nc.sync.dma_start(out=wbs[0], in_=wtb[0])
            nc.sync.dma_start(out=xb[1], in_=xh[1])
            nc.sync.dma_start(out=xb[2], in_=xh[2])
            for h in range(1, R):
                nc.sync.dma_start(out=was[h], in_=wta[h], single_packet=True)
                nc.sync.dma_start(out=wbs[h], in_=wtb[h], single_packet=True)

            # Assemble halo tiles for h = 1, 2, 4, 5 from the base tiles via
            # partition-offset copies on gpsimd (idle otherwise). h=3p+r
            # needs base[p][32r:96] in partitions [0:96-32r] and
            # base[p+1][0:32r] in partitions [96-32r:96].
            xt_of = {0: xb[0], 3: xb[1], 6: xb[2]}
            def _assemble(h):
                p, rr = divmod(h, 3)
                x_t = xasm.tile([KP, JW * N], bf16)
                # one 32-partition copy per halo row (BIR: offset partition
                # windows may span at most 32 partitions); DVE bf16 copies
                # (Act ACTIVATE copies here overload the scalar engine)
                for i in range(3):
                    sp, sr = divmod(rr + i, 3)
                    nc.vector.tensor_copy(
                        out=x_t[32 * i : 32 * (i + 1), :],
                        in_=xb[p + sp][32 * sr : 32 * (sr + 1), :],
                    )
                xt_of[h] = x_t

            for h in range(R):
                x_t = xt_of[h]
                w_a = was[h]
                w_b = wbs[h]
                orow = opool.tile([NG * N, GP * O], bf16)
                for g in range(NG):
                    wa = g * GP
                    ps = pspool.tile([N, GP * O], f32)
                    nc.vector.memset(ps, 0.0)
                    # padded x columns 0 and 57 are all-zero: skipped
                    jlist = [j for j in range(wa, wa + GP + 2) if 0 < j < JW - 1]
                    for j in jlist:
                        lo = max(j - 2, wa)
                        hi = min(j, wa + GP - 1)
                        wlo = lo - (j - 2)
                        nwin = hi - lo + 1
                        if j < JSPLIT:
                            rhs = w_a[:, (j - 1) * 96 + wlo * O :]
                        else:
                            rhs = w_b[:, (j - JSPLIT) * 96 + wlo * O :]
                        nc.tensor.matmul(
                            ps[:, (lo - wa) * O : (lo - wa + nwin) * O],
                            lhsT=x_t[:, j * N : (j + 1) * N],
                            rhs=rhs[:, : nwin * O],
                            start=False,
                            stop=(j == jlist[-1]),
                            skip_group_check=True,
                        )
                    # evict bank g (fp32) as bf16 into the 128-partition row
                    # tile at partition offset 32*g (straight copy, no reorder)
                    nc.scalar.copy(out=orow[g * N : (g + 1) * N, :], in_=ps)
                nc.scalar.dma_start(out=out[h], in_=orow)
                # emit halo assembly after a row's work so the copies overlap
                # that row's matmuls instead of blocking its PSUM inits
                if h == 0:
                    _assemble(1)
                    _assemble(2)
                elif h == 2:
                    _assemble(4)
                    _assemble(5)

    _split_multi_waits(nc)
    _nc_cache = nc
    return nc


def _pack_core(weight, xp, core):
    h0 = core * R
    Wc = weight[:, h0 : h0 + R]  # [O, R, W, C, 3, 3]
    wtc = np.zeros((3, C, R, JW, 3, O), np.float32)
    for wp in range(3):
        k = 2 - wp
        src = Wc[:, :, :, :, :, k]  # [O, R, W, C, I]
        wtc[:, :, :, 2 - wp : 2 - wp + W, wp, :] = src.transpose(4, 3, 1, 2, 0)
    # [R, (i,c), (j, s, o)], border columns j=0 and j=57 dropped; split into
    # two contiguous per-h chunks at j=JSPLIT for fine-grained streaming
    wtc = wtc.transpose(2, 0, 1, 3, 4, 5).reshape(R, KP, JW, 3 * O)
    wta = np.ascontiguousarray(
        wtc[:, :, 1:JSPLIT].reshape(R, KP, -1)
    ).astype(bfloat16)
    wtb = np.ascontiguousarray(
        wtc[:, :, JSPLIT : JW - 1].reshape(R, KP, -1)
    ).astype(bfloat16)
    # x: padded rows h0..h0+8 as three 3-row tiles [(r, c), (j, n)]
    xhc = (
        xp[:, :, h0 : h0 + R + 2, :]
        .transpose(2, 1, 3, 0)
        .reshape(3, KP, JW * N)
    )
    return {
        "wta": wta,
        "wtb": wtb,
        "xh": np.ascontiguousarray(xhc).astype(bfloat16),
    }


def kernel(x, weight, bias, _want_trace=False):
    x = np.asarray(x, dtype=np.float32)
    weight = np.asarray(weight, dtype=np.float32)
    bias = np.asarray(bias, dtype=np.float32)
    nc = _build_nc()
    xp = np.pad(x, ((0, 0), (0, 0), (1, 1), (1, 1)))
    in_maps = [_pack_core(weight, xp, c) for c in range(NCORES)]
    res = run_bass_kernel_spmd(
        nc, in_maps, core_ids=list(range(NCORES)), trace=_want_trace
    )
    outs = []
    for i in range(NCORES):
        o = res.results[i]["out"].astype(np.float32)  # [R, (g, n), (w', o)]
        o = (
            o.reshape(R, NG, N, GP, O)
            .transpose(2, 4, 0, 1, 3)
            .reshape(N, O, R, W)
        )
        outs.append(o)
    full = np.concatenate(outs, axis=2) + bias
    if _want_trace:
        return full, res
    return full
